# revision 36
# baseline (speedup 1.0000x reference)
"""Trainium2 Bass kernel for nn_CGPCoupler (sparse Clebsch-Gordan bilinear coupling).

Reference computation:
    out[:, ro] += x1[:, r1] * x2[:, r2] * cg        (nnz = 9856 sparse entries)

Structure exploited: the index triples come in 16-wide aligned runs, so the whole
op factors over 16-element "subslots" (40 of them in the 640-dim rep space):

    out_O  +=  c_t * (x1_A  (*)  x2_B)      for 616 subslot-triples t=(A,B,O,c)

with only D=308 distinct (A,B) products. Dataflow (per core, data parallel over
the batch dim, 1024 rows/core, fp16 datapath / fp32 PSUM):

    layout:  x2f[p = subslot*2 + ch_half (80 partitions), f = n*8 + ch_lo (8192)]
    host:    x1g = x1 replicated into product-row order (numpy fancy-index),
             streamed straight from HBM (no on-chip gather for side 1)
    1. G2 = SEL2^T @ x2f      (TensorE one-hot selection matmul -> PSUM)
    2. P  = x1g * G2          (VectorE; 4 of 5 chunks evacuated to SBUF fp16 by
                               ScalarE first so the multiply runs in 2x mode)
    3. out = W^T @ P          (TensorE, CG coeffs folded into constant fp16 W,
                               PSUM-accumulated over the 5 product-row chunks)

Host-side numpy work (layout shuffles, building SEL2/W/x1g) is preprocessing of
inputs/constants; all arithmetic combining x1 and x2 happens on the NeuronCores.
"""

import os
import sys
import types

import numpy as np


def _ensure_ntff_hook():
    """concourse's trace path imports antenv.axon_hooks, which this image's
    antenv lacks. Provide it (and register the real profiling hook when the
    axon boot module is available) so tracing works instead of crashing."""
    try:
        import antenv
    except ImportError:
        return
    if getattr(antenv, "axon_hooks", None) is not None:
        return
    try:
        from antenv import axon_hooks  # noqa: F401
        return
    except ImportError:
        pass
    mod = types.ModuleType("antenv.axon_hooks")
    state = {"hook": None}
    mod.set_axon_ntff_profile_hook = lambda h: state.__setitem__("hook", h)
    mod.get_axon_ntff_profile_hook = lambda: state["hook"]
    sys.modules["antenv.axon_hooks"] = mod
    antenv.axon_hooks = mod
    try:
        from trn_agent_boot.trn_boot import _ntff_profile_via_ctypes
        so = "/opt/axon/libaxon_pjrt.so"
        if os.path.exists(so):
            mod.set_axon_ntff_profile_hook(_ntff_profile_via_ctypes(so))
    except Exception:
        pass


_ensure_ntff_hook()

N = 8192
DIM = 640
NCORES = 8
NLOC = N // NCORES          # rows per core
NSUB = DIM // 16            # 40 subslots
P_IN = NSUB * 2             # 80 partitions: (subslot, ch-half)
CHH = 8                     # channels per half
FTOT = NLOC * CHH           # 8192 free elements per partition
FSUP = 2048                 # free-dim super-chunk (per DMA / out tile)
FCH = 512                   # free-dim chunk per matmul (one PSUM bank, fp32)

LAST_RESULTS = None         # BassKernelResults of the most recent run

_matrices_cache = {}
_program_cache = {}


def _build_matrices(cg, r1, r2, ro):
    """Derive subslot terms from the sparse index lists and build the constant
    SEL1/SEL2/W matrices. Everything is validated with asserts."""
    key = (r1.tobytes(), r2.tobytes(), ro.tobytes(), cg.tobytes())
    hit = _matrices_cache.get(key)
    if hit is not None:
        return hit

    A = r1 // 16
    B = r2 // 16
    O = ro // 16
    j = r1 % 16
    assert (r2 % 16 == j).all() and (ro % 16 == j).all(), \
        "index triples are not 16-aligned runs"
    assert A.max() < NSUB and B.max() < NSUB and O.max() < NSUB

    terms = {}   # (A,B,O) -> [coeff, covered-bitmask]
    for a, b, o, jj, c in zip(A.tolist(), B.tolist(), O.tolist(),
                              j.tolist(), cg.tolist()):
        k = (a, b, o)
        e = terms.get(k)
        if e is None:
            terms[k] = [c, 1 << jj]
        else:
            assert e[0] == c, "coefficient varies within a 16-run"
            assert not (e[1] >> jj) & 1, "duplicate (A,B,O,j) entry"
            e[1] |= 1 << jj
    for k, (c, mask) in terms.items():
        assert mask == 0xFFFF, f"term {k} covers only mask {mask:#x}"

    products = sorted({(a, b) for (a, b, o) in terms})
    pidx = {ab: d for d, ab in enumerate(products)}
    D = len(products)
    D2 = 2 * D
    nchunks = (D2 + 127) // 128
    D2p = 128 * nchunks

    SEL2 = np.zeros((P_IN, D2p), np.float16)
    A2 = np.zeros(D2p, np.int64)   # product row -> source row in x1f layout
    W = np.zeros((D2p, P_IN), np.float16)
    for (a, b), d in pidx.items():
        for hh in (0, 1):
            SEL2[b * 2 + hh, 2 * d + hh] = 1.0
            A2[2 * d + hh] = a * 2 + hh
    for (a, b, o), (c, _) in terms.items():
        d = pidx[(a, b)]
        for hh in (0, 1):
            W[2 * d + hh, o * 2 + hh] = c

    # pack W row-chunks side by side: WPACK[:, c*P_IN:(c+1)*P_IN] = W[c*128:...]
    WPACK = np.zeros((128, nchunks * P_IN), np.float16)
    for c in range(nchunks):
        WPACK[:, c * P_IN:(c + 1) * P_IN] = W[c * 128:(c + 1) * 128, :]

    out = (A2, SEL2, WPACK, nchunks)
    _matrices_cache[key] = out
    return out


def _pack_x(x):
    """[NLOC, 640] -> [80, NLOC*8] fp16: row p = subslot*2 + half, col = n*8 + ch."""
    return np.ascontiguousarray(
        x.reshape(NLOC, NSUB, 2, CHH).transpose(1, 2, 0, 3).reshape(P_IN, FTOT),
        dtype=np.float16)


def _unpack_out(o):
    """[80, NLOC*8] -> [NLOC, 640]."""
    return o.reshape(NSUB, 2, NLOC, CHH).transpose(2, 0, 1, 3).reshape(NLOC, DIM)


def _build_program(nchunks):
    """fp16 datapath, v3: the G1 side (x1 replicated into product-row order) is
    prepared on the host and streamed straight from HBM — no gather matmul and
    no PSUM round-trip for it. On-chip work per super-chunk of 1024 free elems:
      - G2 = SEL2^T @ x2f  (TensorE -> PSUM)
      - P[c] = x1g[c] * G2[c]   (VectorE; for NEVAC chunks ScalarE first
        evacuates G2 to SBUF fp16 so the multiply runs in 2x 16-bit mode)
      - out += W[c]^T @ P[c]    (TensorE, PSUM-accumulated)
    """
    import concourse.mybir as mybir
    import concourse.tile as tile
    from concourse import bacc
    from concourse.bass import ds, ts

    f32 = mybir.dt.float32
    f16 = mybir.dt.float16
    nc = bacc.Bacc("TRN2", target_bir_lowering=False)

    FSUP_ = 1024            # free-dim super-chunk
    NSUP = FTOT // FSUP_    # 8
    NJ = FSUP_ // FCH       # 2 matmul FD chunks per super-chunk
    NEVAC = 4               # chunks whose G2 is evacuated by ScalarE (2x TT on V)

    x1gd = nc.dram_tensor("x1g", [nchunks, 128, FTOT], f16, kind="ExternalInput")
    x2d = nc.dram_tensor("x2f", [P_IN, FTOT], f16, kind="ExternalInput")
    s2d = nc.dram_tensor("sel2", [P_IN, nchunks * 128], f16, kind="ExternalInput")
    wd = nc.dram_tensor("wmat", [128, nchunks * P_IN], f16, kind="ExternalInput")
    outd = nc.dram_tensor("outf", [P_IN, FTOT], f32, kind="ExternalOutput")

    with tile.TileContext(nc) as tc:
        with tc.tile_pool(name="const", bufs=1) as constp, \
             tc.tile_pool(name="x1io", bufs=3 * nchunks) as x1io, \
             tc.tile_pool(name="x2io", bufs=3) as x2io, \
             tc.tile_pool(name="gsb", bufs=4) as gsb, \
             tc.tile_pool(name="psb", bufs=2 * nchunks) as psb, \
             tc.tile_pool(name="og", bufs=3) as og, \
             tc.tile_pool(name="psg", bufs=3, space="PSUM") as psg, \
             tc.tile_pool(name="pso", bufs=2, space="PSUM") as pso:

            s2 = constp.tile([P_IN, nchunks * 128], f16, tag="s2")
            nc.scalar.dma_start(out=s2, in_=s2d[:])
            w = constp.tile([128, nchunks * P_IN], f16, tag="w")
            nc.scalar.dma_start(out=w, in_=wd[:])

            for sup in range(NSUP):
                ssl = ds(sup * FSUP_, FSUP_)
                x2t = x2io.tile([P_IN, FSUP_], f16, tag="x2t")
                # SWDGE (GpSimd) queue: keeps ScalarE free for evacuations
                nc.gpsimd.dma_start(out=x2t, in_=x2d[:, ssl])
                x1gt = []
                for c in range(nchunks):
                    t = x1io.tile([128, FSUP_], f16, tag="x1g")
                    # the big streaming input gets its own (sync) HWDGE queue
                    nc.sync.dma_start(
                        out=t, in_=x1gd[c, :, sup * FSUP_:(sup + 1) * FSUP_])
                    x1gt.append(t)

                pts = []
                for c in range(nchunks):
                    g2p = psg.tile([128, FSUP_], f32, tag="gp")
                    for j in range(NJ):
                        nc.tensor.matmul(g2p[:, ts(j, FCH)], s2[:, ts(c, 128)],
                                         x2t[:, ts(j, FCH)], start=True, stop=True)
                    pt = psb.tile([128, FSUP_], f16, tag="pt")
                    if c >= nchunks - NEVAC:
                        # evacuated chunks: TT runs in 2x 16-bit mode; kept last
                        # so the scatter isn't gated by the slow psum-read TT
                        # (chunk 0's 1x TT hides under the remaining gathers)
                        g2s = gsb.tile([128, FSUP_], f16, tag="g2s")
                        nc.scalar.copy(out=g2s, in_=g2p)
                        nc.vector.tensor_mul(pt, x1gt[c], g2s)
                    else:
                        nc.vector.tensor_mul(pt, x1gt[c], g2p)
                    pts.append(pt)

                # scatter: W[c] PSUM-accumulated over c, one bank per j
                outps = []
                for j in range(NJ):
                    outp_j = pso.tile([P_IN, FCH], f32, tag="outp")
                    outps.append(outp_j)
                for c in range(nchunks):
                    for j in range(NJ):
                        nc.tensor.matmul(outps[j], w[:, ts(c, P_IN)],
                                         pts[c][:, ts(j, FCH)],
                                         start=(c == 0), stop=(c == nchunks - 1),
                                         skip_group_check=True)
                outt = og.tile([P_IN, FSUP_], f32, tag="outt")
                if sup == NSUP - 1:
                    # final super-chunk is the kernel tail: parallelize the two
                    # copies across V/S and ship via the low-latency HWDGE path
                    nc.vector.tensor_copy(out=outt[:, ts(0, FCH)], in_=outps[0])
                    nc.scalar.copy(out=outt[:, ts(1, FCH)], in_=outps[1])
                    nc.scalar.dma_start(out=outd[:, ssl], in_=outt)
                else:
                    for j in range(NJ):
                        nc.vector.tensor_copy(out=outt[:, ts(j, FCH)], in_=outps[j])
                    nc.gpsimd.dma_start(out=outd[:, ssl], in_=outt)
    nc.compile()
    return nc


def kernel(x1, x2, cg_tilde, repids_in1, repids_in2, repids_out, out_dim=DIM,
           **_ignored):
    global LAST_RESULTS
    import concourse.bass_utils as _bu
    from concourse.bass_utils import run_bass_kernel_spmd
    # the trace path uploads artifacts to S3, which this container can't reach
    if not getattr(_bu.upload_artifacts, "_local", False):
        _bu.upload_artifacts = lambda tmpdir: "local://" + tmpdir
        _bu.upload_artifacts._local = True

    x1 = np.ascontiguousarray(np.asarray(x1), dtype=np.float32)
    x2 = np.ascontiguousarray(np.asarray(x2), dtype=np.float32)
    cg = np.asarray(cg_tilde, dtype=np.float32)
    r1 = np.asarray(repids_in1, dtype=np.int64)
    r2 = np.asarray(repids_in2, dtype=np.int64)
    ro = np.asarray(repids_out, dtype=np.int64)
    out_dim = int(out_dim)
    assert x1.shape == (N, DIM) and x2.shape == (N, DIM) and out_dim == DIM

    A2, SEL2, WPACK, nchunks = _build_matrices(cg, r1, r2, ro)

    nc = _program_cache.get(nchunks)
    if nc is None:
        nc = _build_program(nchunks)
        _program_cache[nchunks] = nc

    in_maps = []
    for c in range(NCORES):
        sl = slice(c * NLOC, (c + 1) * NLOC)
        x1f = _pack_x(x1[sl])
        in_maps.append({
            "x1g": np.ascontiguousarray(
                x1f[A2].reshape(nchunks, 128, FTOT)),
            "x2f": _pack_x(x2[sl]),
            "sel2": SEL2,
            "wmat": WPACK,
        })

    res = run_bass_kernel_spmd(nc, in_maps, core_ids=list(range(NCORES)))
    LAST_RESULTS = res

    out = np.empty((N, DIM), np.float32)
    for c in range(NCORES):
        out[c * NLOC:(c + 1) * NLOC] = _unpack_out(
            np.asarray(res.results[c]["outf"], dtype=np.float32))
    return out


def _numpy_model(x1, x2, cg, r1, r2, ro):
    """Host-side model of the device dataflow (including fp16 quantization),
    for validating index logic and predicting the on-device error."""
    A2, SEL2, WPACK, nchunks = _build_matrices(cg, r1, r2, ro)
    W = np.zeros((128 * nchunks, P_IN), np.float32)
    for c in range(nchunks):
        W[c * 128:(c + 1) * 128, :] = WPACK[:, c * P_IN:(c + 1) * P_IN].astype(
            np.float32)
    out = np.empty_like(x1)
    for c in range(NCORES):
        sl = slice(c * NLOC, (c + 1) * NLOC)
        x1f = _pack_x(x1[sl])
        x2f = _pack_x(x2[sl]).astype(np.float32)
        g1 = x1f[A2].astype(np.float32)
        g2 = (SEL2.astype(np.float32).T @ x2f).astype(np.float16)  # worst branch
        p = (g1 * g2.astype(np.float32)).astype(np.float16)
        outf = W.T @ p.astype(np.float32)
        out[sl] = _unpack_out(outf)
    return out
